# revision 1
# baseline (speedup 1.0000x reference)
"""NeuronMemory retrieval kernel for 8 TRN2 NeuronCores.

Problem (hardcoded shapes):
  x                [2, 2048, 1024] f32
  router_w         [16, 1024] f32
  compress_neurons [16, 1024, 128] f32
  knowledge_K      [32768, 128] f32
  knowledge_V      [32768, 1024] f32
  out              [2, 2048, 1024] f32

Per token: softmax-routed low-rank projection Q (rank 128), dense scores
against 32768 knowledge keys, top-8, softmax, weighted gather of V rows.

Sharding: data-parallel over the 4096 tokens (512 tokens/core); router,
compress_neurons, knowledge tables replicated on every core. No collectives.

Per-core pipeline (4 token tiles of 128):
  A. router scores + softmax -> wts [128, 16]
  B. Q = sum_n wts_n * (x @ W_n) via grouped matmuls + fused weighting,
     PE-transpose -> QT [r=128, tokens], scale by 1/sqrt(128)
  C. scores = QT.T @ KT in 512-col chunks -> PSUM -> SBUF; per 8192-quarter
     hardware top-8 (max) + indices (max_index)
  D. merge 32 candidates/token -> exact top-8 + global indices
  E. softmax over top-8
  F. indirect-DMA gather of V rows + fused weighted accumulate -> out
"""
import numpy as np

import concourse.bacc as bacc
import concourse.bass as bass
import concourse.mybir as mybir
from concourse.tile import TileContext
from concourse.bass_utils import run_bass_kernel_spmd

P = 128
D_MODEL = 1024
RANK = 128
N_COMPRESS = 16
N_KNOWLEDGE = 32768
K_TOP = 8
B, S = 2, 2048
N_CORES = 8
TOK_PER_CORE = (B * S) // N_CORES      # 512
N_TILES = TOK_PER_CORE // P            # 4
N_DC = D_MODEL // P                    # 8 d-model chunks
N_Q = 4                                # knowledge quarters
QW = N_KNOWLEDGE // N_Q                # 8192 quarter width
N_CH = QW // 512                       # 16 chunks of 512 per quarter
N_G = 4                                # neuron groups of 4
SCALE = 1.0 / np.sqrt(np.float32(RANK))

f32 = mybir.dt.float32
u32 = mybir.dt.uint32


def _build(dbg=False):
    nc = bacc.Bacc("TRN2", target_bir_lowering=False, debug=False, num_devices=N_CORES)

    xT = nc.declare_dram_parameter("xT", [P, N_DC * TOK_PER_CORE], f32, isOutput=False)
    rw = nc.declare_dram_parameter("rw", [P, N_DC * N_COMPRESS], f32, isOutput=False)
    Wg = nc.declare_dram_parameter("Wg", [N_G * N_DC * P, 512], f32, isOutput=False)
    KT = nc.declare_dram_parameter("KT", [P, N_KNOWLEDGE], f32, isOutput=False)
    V0 = nc.declare_dram_parameter("V0", [N_KNOWLEDGE, 512], f32, isOutput=False)
    V1 = nc.declare_dram_parameter("V1", [N_KNOWLEDGE, 512], f32, isOutput=False)
    ident = nc.declare_dram_parameter("ident", [P, P], f32, isOutput=False)
    out = nc.declare_dram_parameter("out", [TOK_PER_CORE, D_MODEL], f32, isOutput=True)
    if dbg:
        d_wts = nc.declare_dram_parameter("d_wts", [P, N_TILES * N_COMPRESS], f32, isOutput=True)
        d_q = nc.declare_dram_parameter("d_q", [P, N_TILES * RANK], f32, isOutput=True)
        d_cv = nc.declare_dram_parameter("d_cv", [P, N_TILES * N_Q * 8], f32, isOutput=True)
        d_cif = nc.declare_dram_parameter("d_cif", [P, N_TILES * N_Q * 8], f32, isOutput=True)
        d_v8 = nc.declare_dram_parameter("d_v8", [P, N_TILES * 8], f32, isOutput=True)
        d_idx = nc.declare_dram_parameter("d_idx", [P, N_TILES * 8], f32, isOutput=True)
        d_w8 = nc.declare_dram_parameter("d_w8", [P, N_TILES * 8], f32, isOutput=True)
        d_gat = nc.declare_dram_parameter("d_gat", [P, K_TOP * 512], f32, isOutput=True)

    Wg_v = Wg.rearrange("(g dc p) n -> g dc p n", g=N_G, dc=N_DC)

    with TileContext(nc) as tc:
        with (
            tc.tile_pool(name="const", bufs=1) as cpool,
            tc.tile_pool(name="kt", bufs=2) as ktpool,
            tc.tile_pool(name="sc", bufs=2) as scpool,
            tc.tile_pool(name="wld", bufs=3) as wpool,
            tc.tile_pool(name="gat", bufs=2) as gpool,
            tc.tile_pool(name="acc", bufs=2) as apool,
            tc.tile_pool(name="small", bufs=4) as spool,
            tc.tile_pool(name="ps_big", bufs=4, space="PSUM") as psb,
            tc.tile_pool(name="ps_small", bufs=2, space="PSUM") as pss,
        ):
            # ---- persistent loads ----
            xT_sb = cpool.tile([P, N_DC * TOK_PER_CORE], f32)   # 16KB/part
            rw_sb = cpool.tile([P, N_DC * N_COMPRESS], f32)
            id_sb = cpool.tile([P, P], f32)
            nc.sync.dma_start(out=xT_sb[:], in_=xT[:])
            nc.sync.dma_start(out=rw_sb[:], in_=rw[:])
            nc.sync.dma_start(out=id_sb[:], in_=ident[:])

            wts_sb = cpool.tile([P, N_TILES * N_COMPRESS], f32)  # router weights per tile
            Q_sb = cpool.tile([P, N_TILES * RANK], f32)          # [tokens, r] per tile
            QT_sb = cpool.tile([P, N_TILES * P], f32)            # [r, tokens] per tile
            cv_sb = cpool.tile([P, N_TILES * N_Q * 8], f32)      # candidate values
            cif_sb = cpool.tile([P, N_TILES * N_Q * 8], f32)     # candidate idx as f32

            def tok(t):
                return slice(t * P, (t + 1) * P)

            # ---- A: router softmax ----
            for t in range(N_TILES):
                rps = pss.tile([P, N_COMPRESS], f32, space="PSUM", tag="rps")
                for dc in range(N_DC):
                    nc.tensor.matmul(
                        out=rps[:],
                        lhsT=xT_sb[:, dc * TOK_PER_CORE + t * P:dc * TOK_PER_CORE + (t + 1) * P],
                        rhs=rw_sb[:, dc * N_COMPRESS:(dc + 1) * N_COMPRESS],
                        start=(dc == 0), stop=(dc == N_DC - 1),
                    )
                w = wts_sb[:, t * N_COMPRESS:(t + 1) * N_COMPRESS]
                mx = spool.tile([P, 1], f32, tag="mx")
                sm = spool.tile([P, 1], f32, tag="sm")
                ex = spool.tile([P, N_COMPRESS], f32, tag="ex")
                nc.vector.tensor_reduce(out=mx[:], in_=rps[:], op=mybir.AluOpType.max, axis=mybir.AxisListType.X)
                nc.vector.tensor_scalar(out=ex[:], in0=rps[:], scalar1=mx[:, :1], scalar2=None, op0=mybir.AluOpType.subtract)
                nc.scalar.activation(out=ex[:], in_=ex[:], func=mybir.ActivationFunctionType.Exp,
                                     accum_out=sm[:, :1])
                rcp = spool.tile([P, 1], f32, tag="rcp")
                nc.vector.reciprocal(out=rcp[:], in_=sm[:, :1])
                nc.vector.tensor_scalar(out=w, in0=ex[:], scalar1=rcp[:, :1], scalar2=None, op0=mybir.AluOpType.mult)

            # ---- B: Q projection ----
            # Q[t] accumulated over groups; per group g: Y[t] = x @ W_g (4 neurons wide)
            yps_tiles = {}
            for g in range(N_G):
                for dc in range(N_DC):
                    wtile = wpool.tile([P, 512], f32, tag="wld")
                    nc.sync.dma_start(out=wtile[:], in_=Wg_v[g, dc])
                    for t in range(N_TILES):
                        if dc == 0:
                            yps_tiles[t] = psb.tile([P, 512], f32, space="PSUM", tag="ps", name=f"yps_{g}_{t}")
                        nc.tensor.matmul(
                            out=yps_tiles[t][:],
                            lhsT=xT_sb[:, dc * TOK_PER_CORE + t * P:dc * TOK_PER_CORE + (t + 1) * P],
                            rhs=wtile[:],
                            start=(dc == 0), stop=(dc == N_DC - 1),
                        )
                for t in range(N_TILES):
                    q = Q_sb[:, t * RANK:(t + 1) * RANK]
                    for n in range(4):
                        ncomp = g * 4 + n
                        wcol = wts_sb[:, t * N_COMPRESS + ncomp:t * N_COMPRESS + ncomp + 1]
                        ypart = yps_tiles[t][:, n * RANK:(n + 1) * RANK]
                        if g == 0 and n == 0:
                            nc.vector.tensor_scalar(out=q, in0=ypart, scalar1=wcol, scalar2=None,
                                                    op0=mybir.AluOpType.mult)
                        else:
                            nc.vector.scalar_tensor_tensor(out=q, in0=ypart, scalar=wcol, in1=q,
                                                           op0=mybir.AluOpType.mult,
                                                           op1=mybir.AluOpType.add)

            # transpose Q -> QT, apply 1/sqrt(RANK)
            for t in range(N_TILES):
                tps = pss.tile([P, P], f32, space="PSUM", tag="tps")
                nc.tensor.transpose(out=tps[:], in_=Q_sb[:, t * RANK:(t + 1) * RANK], identity=id_sb[:])
                nc.scalar.activation(out=QT_sb[:, tok(t)], in_=tps[:],
                                     func=mybir.ActivationFunctionType.Copy, scale=float(SCALE))

            # ---- C: knowledge scores + per-quarter top8 ----
            for q in range(N_Q):
                ktq = ktpool.tile([P, QW], f32, tag="ktq")
                nc.sync.dma_start(out=ktq[:], in_=KT[:, q * QW:(q + 1) * QW])
                for t in range(N_TILES):
                    sc = scpool.tile([P, QW], f32, tag="sc")
                    for c in range(N_CH):
                        sps = psb.tile([P, 512], f32, space="PSUM", tag="ps")
                        nc.tensor.matmul(
                            out=sps[:],
                            lhsT=QT_sb[:, tok(t)],
                            rhs=ktq[:, c * 512:(c + 1) * 512],
                            start=True, stop=True,
                        )
                        nc.scalar.copy(out=sc[:, c * 512:(c + 1) * 512], in_=sps[:])
                    vq = cv_sb[:, (t * N_Q + q) * 8:(t * N_Q + q + 1) * 8]
                    iq = spool.tile([P, 8], u32, tag="iq")
                    nc.vector.max(out=vq, in_=sc[:])
                    nc.vector.max_index(out=iq[:], in_max=vq, in_values=sc[:])
                    # global index = local + q*QW, keep as f32 (exact below 2^24)
                    nc.vector.tensor_copy(out=cif_sb[:, (t * N_Q + q) * 8:(t * N_Q + q + 1) * 8], in_=iq[:])
                    if q > 0:
                        nc.vector.tensor_scalar(
                            out=cif_sb[:, (t * N_Q + q) * 8:(t * N_Q + q + 1) * 8],
                            in0=cif_sb[:, (t * N_Q + q) * 8:(t * N_Q + q + 1) * 8],
                            scalar1=float(q * QW), scalar2=None, op0=mybir.AluOpType.add)

            if dbg:
                nc.sync.dma_start(out=d_wts[:], in_=wts_sb[:])
                nc.sync.dma_start(out=d_q[:], in_=Q_sb[:])
                nc.sync.dma_start(out=d_cv[:], in_=cv_sb[:])
                nc.sync.dma_start(out=d_cif[:], in_=cif_sb[:])

            # ---- D/E/F per tile ----
            NCAND = N_Q * 8
            for t in range(N_TILES):
                cv = cv_sb[:, t * NCAND:(t + 1) * NCAND]
                cif = cif_sb[:, t * NCAND:(t + 1) * NCAND]
                v8 = spool.tile([P, 8], f32, tag="v8")
                nc.vector.max(out=v8[:], in_=cv)
                # resolve global indices: idxf[j] = sum((cv == v8[j]) * cif)
                idxf = spool.tile([P, 8], f32, tag="idxf")
                junk = spool.tile([P, NCAND], f32, tag="junk")
                for j in range(K_TOP):
                    nc.vector.scalar_tensor_tensor(
                        out=junk[:], in0=cv, scalar=v8[:, j:j + 1], in1=cif,
                        op0=mybir.AluOpType.is_equal, op1=mybir.AluOpType.mult,
                        accum_out=idxf[:, j:j + 1])
                gidx = spool.tile([P, 8], u32, tag="gidx")
                nc.vector.tensor_copy(out=gidx[:], in_=idxf[:])
                if dbg:
                    nc.sync.dma_start(out=d_v8[:, t * 8:(t + 1) * 8], in_=v8[:])
                    nc.sync.dma_start(out=d_idx[:, t * 8:(t + 1) * 8], in_=idxf[:])

                # softmax over top8
                w8 = spool.tile([P, 8], f32, tag="w8")
                sm8 = spool.tile([P, 1], f32, tag="sm8")
                nc.vector.tensor_scalar(out=w8[:], in0=v8[:], scalar1=v8[:, :1], scalar2=None,
                                        op0=mybir.AluOpType.subtract)
                nc.scalar.activation(out=w8[:], in_=w8[:], func=mybir.ActivationFunctionType.Exp,
                                     accum_out=sm8[:, :1])
                rcp8 = spool.tile([P, 1], f32, tag="rcp8")
                nc.vector.reciprocal(out=rcp8[:], in_=sm8[:, :1])
                nc.vector.tensor_scalar(out=w8[:], in0=w8[:], scalar1=rcp8[:, :1], scalar2=None,
                                        op0=mybir.AluOpType.mult)
                if dbg:
                    nc.sync.dma_start(out=d_w8[:, t * 8:(t + 1) * 8], in_=w8[:])

                # gather V rows (two 512-wide halves) + weighted accumulate
                for dh, Vh in ((0, V0), (1, V1)):
                    gat = gpool.tile([P, K_TOP * 512], f32, tag="gat")
                    for j in range(K_TOP):
                        nc.gpsimd.indirect_dma_start(
                            out=gat[:, j * 512:(j + 1) * 512],
                            out_offset=None,
                            in_=Vh[:],
                            in_offset=bass.IndirectOffsetOnAxis(ap=gidx[:, j:j + 1], axis=0),
                        )
                    if dbg and t == 0 and dh == 0:
                        nc.sync.dma_start(out=d_gat[:], in_=gat[:])
                    acc = apool.tile([P, 512], f32, tag="acc")
                    nc.vector.tensor_scalar(out=acc[:], in0=gat[:, 0:512], scalar1=w8[:, 0:1],
                                            scalar2=None, op0=mybir.AluOpType.mult)
                    for j in range(1, K_TOP):
                        nc.vector.scalar_tensor_tensor(
                            out=acc[:], in0=gat[:, j * 512:(j + 1) * 512], scalar=w8[:, j:j + 1],
                            in1=acc[:], op0=mybir.AluOpType.mult, op1=mybir.AluOpType.add)
                    nc.sync.dma_start(out=out[t * P:(t + 1) * P, dh * 512:(dh + 1) * 512], in_=acc[:])

    nc.compile()
    return nc


_NC_CACHE = {}


def _get_nc(dbg=False):
    if dbg not in _NC_CACHE:
        _NC_CACHE[dbg] = _build(dbg)
    return _NC_CACHE[dbg]


def _prep_in_maps(x, router_w, compress_neurons, knowledge_K, knowledge_V):
    x = np.asarray(x, dtype=np.float32).reshape(B * S, D_MODEL)
    rwT = np.ascontiguousarray(np.asarray(router_w, dtype=np.float32).T)      # [1024, 16]
    rw_r = np.ascontiguousarray(
        rwT.reshape(N_DC, P, N_COMPRESS).transpose(1, 0, 2).reshape(P, N_DC * N_COMPRESS))
    cn = np.asarray(compress_neurons, dtype=np.float32)
    Wg = np.ascontiguousarray(
        cn.reshape(N_G, 4, N_DC, P, RANK).transpose(0, 2, 3, 1, 4).reshape(N_G * N_DC * P, 4 * RANK))
    KT = np.ascontiguousarray(np.asarray(knowledge_K, dtype=np.float32).T)    # [128, 32768]
    V = np.asarray(knowledge_V, dtype=np.float32)
    V0 = np.ascontiguousarray(V[:, :512])
    V1 = np.ascontiguousarray(V[:, 512:])
    ident = np.eye(P, dtype=np.float32)

    in_maps = []
    for c in range(N_CORES):
        xs = x[c * TOK_PER_CORE:(c + 1) * TOK_PER_CORE]                        # [512, 1024]
        xT = np.ascontiguousarray(
            xs.T.reshape(N_DC, P, TOK_PER_CORE).transpose(1, 0, 2).reshape(P, N_DC * TOK_PER_CORE))
        in_maps.append(dict(xT=xT, rw=rw_r, Wg=Wg, KT=KT, V0=V0, V1=V1, ident=ident))
    return in_maps


def _ensure_ntff_hook():
    import sys as _sys
    import types as _types
    if "antenv.axon_hooks" in _sys.modules:
        return
    try:
        import antenv.axon_hooks  # noqa: F401
        return
    except ImportError:
        pass
    mod = _types.ModuleType("antenv.axon_hooks")
    _state = {"hook": None}
    mod.set_axon_ntff_profile_hook = lambda h: _state.__setitem__("hook", h)
    mod.get_axon_ntff_profile_hook = lambda: _state["hook"]
    _sys.modules["antenv.axon_hooks"] = mod
    try:
        from trn_agent_boot.trn_boot import _ntff_profile_via_ctypes
        mod.set_axon_ntff_profile_hook(_ntff_profile_via_ctypes("/opt/axon/libaxon_pjrt.so"))
    except Exception:
        pass


def _run(inputs, trace=False, dbg=False):
    if trace:
        _ensure_ntff_hook()
    nc = _get_nc(dbg)
    in_maps = _prep_in_maps(**inputs)
    res = run_bass_kernel_spmd(nc, in_maps, core_ids=list(range(N_CORES)), trace=trace)
    out = np.concatenate([res.results[c]["out"] for c in range(N_CORES)], axis=0)
    return out.reshape(B, S, D_MODEL), res


def kernel(x, router_w, compress_neurons, knowledge_K, knowledge_V):
    out, _ = _run(dict(x=x, router_w=router_w, compress_neurons=compress_neurons,
                       knowledge_K=knowledge_K, knowledge_V=knowledge_V))
    return out

